# revision 34
# baseline (speedup 1.0000x reference)
"""Trainium2 Bass kernel for the box-ranking depth loss (v2).

Structure (vs the v1 prefix-scan kernel):
  - Sums/sumsq per box now run on the idle PE engine: the host ships a
    column-transposed fp16 slab dsT[p, k*128+r] = d[r, 128k+p] plus a
    per-chunk column-indicator colind[p, k*32+t]; 16 fp16 matmuls
    accumulate per-(row,box) sums in PSUM, an ACT square + 16 more
    matmuls give sums of squares.  This removes both DVE prefix scans
    and the 64 per-box prefix-difference ops.
  - Min/max sliding tables are all-fp16 (host ships dsh = fp16(d) and
    dsn = fp16(-d); min side runs as MAX on dsn) and stop at width 8
    (3 levels per side).  Each per-box lookup is ONE fused
    tensor_tensor_reduce over a 4-arm strided window view (width-32
    windows covering [x1,x2), each window = max of four h8 entries).
  - Cross-partition combines use gpsimd partition_all_reduce(max) on the
    Pool engine instead of PE-transpose + reduce.
  - Two collectives: an early AllGather carrying box sums/sumsq plus the
    core-local global -min/max (so the whole loss_acc / mean / std
    pipeline hides under the min/max table work), and one late AllGather
    with the per-box -min/max.  The late readback lands boxes on
    partitions with (-min, max) in adjacent free columns so the final
    range/reciprocal/std math is lane-aligned.

Sharding: rows (H) split 8 ways -> each core holds 128 rows.  Every core
computes the final 3-float result redundantly; the host reads core 0.
"""

import numpy as np

H, W, T, NCORES = 1024, 2048, 32, 8
R = H // NCORES  # 128 rows per core
BIG = 1e30
RATIO = 1.0
NCHUNK = 16  # 2048 cols / 128
DINH_W = 2 * W + W + 32 * NCHUNK          # dsn | dsh | dsT | colind
CST_W = 292
SUMS_N = 66   # 32 sums | 32 sumsq | -gmin | gmax


def _win_params(x1, x2):
    """Width-32 windows covering [x1, x2): n windows, two interleaved
    arithmetic progressions offset by s1 (s1 == 0 -> single AP)."""
    q = (x2 - x1) - 32
    n = q // 32 + 1
    s1 = q - 32 * (n - 1)
    return n, s1


USE_TTR = False


def _build_program(bboxes, single_core=False, reps=1, mock_cc=False):
    import concourse.bacc as bacc
    import concourse.mybir as mybir
    import concourse.tile as tile
    import concourse.bass_isa as bass_isa
    from concourse.ap import AP
    from concourse.alu_op_type import AluOpType as alu

    f32 = mybir.dt.float32
    f16 = mybir.dt.float16
    X = mybir.AxisListType.X
    XYZWC = mybir.AxisListType.XYZWC
    AF = mybir.ActivationFunctionType
    RO = bass_isa.ReduceOp

    x1s, x2s = bboxes[:, 0], bboxes[:, 2]

    nc = bacc.Bacc("TRN2", target_bir_lowering=False, debug=False,
                   num_devices=1 if single_core else NCORES)

    dinH = nc.dram_tensor("dinH", [R, DINH_W], f16, kind="ExternalInput").ap()
    cst = nc.dram_tensor("cst", [128, CST_W], f32, kind="ExternalInput").ap()
    out = nc.dram_tensor("out", [3], f32, kind="ExternalOutput").ap()

    def sb(name, shape, dt=f32):
        return nc.alloc_sbuf_tensor(name, shape, dt).ap()

    dsn = sb("dsn", [R, W], f16)
    dsh = sb("dsh", [R, W], f16)
    dsT = sb("dsT", [R, W], f16)
    dsq = sb("dsq", [R, W], f16)
    cold = sb("cold", [R, 32 * NCHUNK], f16)
    hn2 = sb("hn2", [R, W], f16)
    hn4 = sb("hn4", [R, W], f16)
    hn8 = sb("hn8", [R, W], f16)
    hn16 = sb("hn16", [R, W], f16)
    hn32 = sb("hn32", [R, W], f16)
    g2 = sb("g2", [R, W], f16)
    g4 = sb("g4", [R, W], f16)
    g8 = sb("g8", [R, W], f16)
    g16 = sb("g16", [R, W], f16)
    g32 = sb("g32", [R, W], f16)
    cstS = sb("cstS", [128, CST_W])
    scr = sb("scr", [R, 128], f16)       # TTR elementwise scratch
    rmm = sb("rmm", [R, 2 * T])          # -min | max lookup accums (f32)
    stk = sb("stk", [R, 2 * T])
    rrs = sb("rrs", [R, T])
    rrs2 = sb("rrs2", [R, T])
    svS = sb("svS", [2 * T, 1])
    gfix = sb("gfix", [R, 2])            # per-row -gmin | gmax
    gg2 = sb("gg2", [2, 1])              # core-local -gmin | gmax (column)
    bmv = sb("bmv", [2 * T, 1])
    gthS = sb("gthS", [SUMS_N, NCORES])
    scrA = sb("scrA", [2 * T, NCORES])
    sv2 = sb("sv2", [2 * T, 1])
    ggrow = sb("ggrow", [1, 2 * NCORES])
    ggred = sb("ggred", [1, 2])
    aden = sb("aden", [1, 1])
    arecip = sb("arecip", [1, 1])
    acolS = sb("acolS", [T, 1])
    meanv = sb("meanv", [T, 1])
    m2sv = sb("m2sv", [T, 1])
    varv = sb("varv", [T, 1])
    stdv = sb("stdv", [T, 1])
    meanTS = sb("meanTS", [1, T])
    qm = sb("qm", [T, T])
    t2m = sb("t2m", [T, T])
    t3m = sb("t3m", [T, T])
    raccv = sb("raccv", [T, 1])
    gtm2 = sb("gtm2", [T, 2 * NCORES])
    redM = sb("redM", [T, 2])
    rngv = sb("rngv", [T, 1])
    rinv = sb("rinv", [T, 1])
    srv = sb("srv", [T, 1])
    dummy = sb("dmy0", [1, 8])
    out3 = sb("out3", [1, 3])

    # const views
    identC = cstS[:, 0:128]
    ident32C = cstS[0:T, 0:T]
    gmatC = cstS[0:T, 128:160]
    cntinvC = cstS[0:T, 160:161]
    cm1invC = cstS[0:T, 161:162]
    ones128C = cstS[:, 162:163]
    ones32C = cstS[0:T, 162:163]
    onesrowC = cstS[0:1, 163:163 + T]
    rinfx2C = cstS[:, 196:260]
    rindC = cstS[:, 260:292]

    def box_ttr(tab, t, x1, x2, accum):
        """One fused lookup: width-32 windows over [x1,x2), each window =
        max of 4 width-8 table entries; reduce-max into accum."""
        n, s1 = _win_params(x1, x2)
        base = tab[:, 0:1]
        ppair = list(base.ap[0])
        s0 = scr[:, 0:1]
        spp = list(s0.ap[0])
        if s1 == 0:
            in0 = AP(base.tensor, base.offset + x1, [ppair, [32, n]])
            in1 = AP(base.tensor, base.offset + x1 + 16, [ppair, [32, n]])
            o = AP(s0.tensor, s0.offset, [spp, [1, n]])
        else:
            in0 = AP(base.tensor, base.offset + x1,
                     [ppair, [s1, 2], [32, n]])
            in1 = AP(base.tensor, base.offset + x1 + 16,
                     [ppair, [s1, 2], [32, n]])
            o = AP(s0.tensor, s0.offset, [spp, [n, 2], [1, n]])
        nc.vector.tensor_tensor_reduce(
            out=o, in0=in0, in1=in1, scale=1.0, scalar=float(-BIG),
            op0=alu.max, op1=alu.max, accum_out=accum)

    def box_red(tab, x1, x2, accum):
        """Baseline-style lookup: one strided reduce over width-32 windows
        of a width-32 table covering [x1, x2)."""
        n, s1 = _win_params(x1, x2)
        base = tab[:, 0:1]
        ppair = list(base.ap[0])
        if s1 == 0:
            v = AP(base.tensor, base.offset + x1, [ppair, [32, n]])
            ax = X
        else:
            v = AP(base.tensor, base.offset + x1, [ppair, [s1, 2], [32, n]])
            ax = mybir.AxisListType.XY
        nc.vector.tensor_reduce(accum, v, ax, alu.max)

    with tile.TileContext(nc) as tc:
        with tc.tile_pool(name="psum", bufs=1, space="PSUM") as pp, \
                tc.tile_pool(name="dram", bufs=1, space="DRAM") as dram:
            rowsum = pp.tile([R, T], f32, name="rowsum")
            rowsq = pp.tile([R, T], f32, name="rowsq")
            psum_s = pp.tile([2 * T, 1], f32, name="psum_s")
            meanT_p = pp.tile([1, T], f32, name="meanT_p")
            ggT = pp.tile([2, 128], f32, name="ggT")
            stkTn = pp.tile([T, 128], f32, name="stkTn")
            stkTx = stkTn
            mr_p = pp.tile([T, T], f32, name="mr_p")
            pl2 = pp.tile([1, 2], f32, name="pl2")

            cstatS = dram.tile([1, SUMS_N], f32, name="cstatS")
            cgathS = dram.tile([NCORES, SUMS_N], f32, name="cgathS")
            cstatN = dram.tile([1, T], f32, name="cstatN")
            cgathN = dram.tile([NCORES, T], f32, name="cgathN")
            cstatX = dram.tile([1, T], f32, name="cstatX")
            cgathX = dram.tile([NCORES, T], f32, name="cgathX")

            def mock_gather(cstat, cgath, nfree, queue):
                q = nc.sync if queue == 0 else nc.scalar
                q.dma_start(out=cgath[:], in_=cstat[0:1, :].broadcast_to(
                    (NCORES, nfree)))

            def gather(cstat, cgath, nfree, queue):
                if single_core or mock_cc:
                    mock_gather(cstat, cgath, nfree, queue)
                else:
                    nc.gpsimd.collective_compute(
                        "AllGather", alu.bypass,
                        replica_groups=[list(range(NCORES))],
                        ins=[cstat[:]], outs=[cgath[:]])

            for _rep in range(reps):
                # ---- ACT function-table preloads (hidden under input DMA) --
                nc.vector.memset(dummy[0:1, 0:1], 0.0)
                nc.scalar.activation(dummy[0:1, 1:2], dummy[0:1, 0:1],
                                     AF.Square)
                nc.scalar.activation(dummy[0:1, 2:3], dummy[0:1, 0:1],
                                     AF.Sqrt)
                nc.scalar.activation(dummy[0:1, 3:4], dummy[0:1, 0:1],
                                     AF.Relu)
                nc.scalar.copy(dummy[0:1, 4:5], dummy[0:1, 0:1])

                # ---- loads: dsn quarters first (feed the min chain), then
                # dsT (sums path), dsh (max chain), consts; 2 HWDGE queues --
                Q = W // 4
                hw = W // 2
                nc.sync.dma_start(out=dsn[:, 0:Q], in_=dinH[:, 0:Q])
                nc.scalar.dma_start(out=dsn[:, Q:2 * Q], in_=dinH[:, Q:2 * Q])
                nc.sync.dma_start(out=dsn[:, 2 * Q:3 * Q],
                                  in_=dinH[:, 2 * Q:3 * Q])
                nc.scalar.dma_start(out=dsn[:, 3 * Q:W], in_=dinH[:, 3 * Q:W])
                nc.sync.dma_start(out=dsT[:, 0:hw],
                                  in_=dinH[:, 2 * W:2 * W + hw])
                nc.scalar.dma_start(out=dsT[:, hw:W],
                                    in_=dinH[:, 2 * W + hw:3 * W])
                nc.sync.dma_start(out=dsh[:, 0:hw], in_=dinH[:, W:W + hw])
                nc.scalar.dma_start(out=cold[:], in_=dinH[:, 3 * W:DINH_W])
                nc.sync.dma_start(out=dsh[:, hw:W], in_=dinH[:, W + hw:2 * W])
                nc.scalar.dma_start(out=cstS[:], in_=cst[:])

                # ---- min-side (negated) sliding tables, fp16 2x ----
                for qi in range(4):
                    a = qi * Q
                    b = min((qi + 1) * Q, W - 1)
                    nc.vector.tensor_tensor(hn2[:, a:b], dsn[:, a:b],
                                            dsn[:, a + 1:b + 1], alu.max)
                nc.vector.tensor_tensor(hn4[:, 0:W - 3], hn2[:, 0:W - 3],
                                        hn2[:, 2:W - 1], alu.max)
                nc.vector.tensor_tensor(hn8[:, 0:W - 7], hn4[:, 0:W - 7],
                                        hn4[:, 4:W - 3], alu.max)
                nc.vector.tensor_tensor(hn16[:, 0:W - 15], hn8[:, 0:W - 15],
                                        hn8[:, 8:W - 7], alu.max)
                nc.vector.tensor_tensor(hn32[:, 0:W - 31], hn16[:, 0:W - 31],
                                        hn16[:, 16:W - 15], alu.max)
                nc.vector.tensor_reduce(gfix[:, 0:1], hn16[:, 0:W - 15:16],
                                        X, alu.max)

                # ---- PE sums path (square on ACT, matmuls on PE) ----
                nc.scalar.square(dsq[:], dsT[:])
                for k in range(NCHUNK):
                    nc.tensor.matmul(rowsum[:], dsT[:, 128 * k:128 * (k + 1)],
                                     cold[:, 32 * k:32 * (k + 1)],
                                     start=(k == 0), stop=(k == NCHUNK - 1))
                for k in range(NCHUNK):
                    nc.tensor.matmul(rowsq[:], dsq[:, 128 * k:128 * (k + 1)],
                                     cold[:, 32 * k:32 * (k + 1)],
                                     start=(k == 0), stop=(k == NCHUNK - 1))
                nc.vector.tensor_tensor(rrs[:], rowsum[:], rindC, alu.mult)
                nc.vector.tensor_tensor(rrs2[:], rowsq[:], rindC, alu.mult)
                nc.tensor.matmul(psum_s[0:T, 0:1], rrs[:], ones128C,
                                 start=True, stop=True)
                nc.tensor.matmul(psum_s[T:2 * T, 0:1], rrs2[:], ones128C,
                                 start=True, stop=True)
                nc.scalar.copy(svS[:], psum_s[:])
                nc.sync.dma_start(out=cstatS[0:1, 0:2 * T], in_=svS[:])

                # ---- max-side sliding tables ----
                nc.vector.tensor_tensor(g2[:, 0:hw - 1], dsh[:, 0:hw - 1],
                                        dsh[:, 1:hw], alu.max)
                nc.vector.tensor_tensor(g2[:, hw - 1:W - 1],
                                        dsh[:, hw - 1:W - 1],
                                        dsh[:, hw:W], alu.max)
                nc.vector.tensor_tensor(g4[:, 0:W - 3], g2[:, 0:W - 3],
                                        g2[:, 2:W - 1], alu.max)
                nc.vector.tensor_tensor(g8[:, 0:W - 7], g4[:, 0:W - 7],
                                        g4[:, 4:W - 3], alu.max)
                nc.vector.tensor_tensor(g16[:, 0:W - 15], g8[:, 0:W - 15],
                                        g8[:, 8:W - 7], alu.max)
                nc.vector.tensor_tensor(g32[:, 0:W - 31], g16[:, 0:W - 31],
                                        g16[:, 16:W - 15], alu.max)
                nc.vector.tensor_reduce(gfix[:, 1:2], g16[:, 0:W - 15:16],
                                        X, alu.max)
                # core-local global -min/max across partitions (PE + DVE)
                nc.tensor.transpose(ggT[:], gfix[:], identC)
                nc.vector.tensor_reduce(gg2[:], ggT[:], X, alu.max)
                nc.scalar.dma_start(out=cstatS[0:1, 2 * T:SUMS_N],
                                    in_=gg2[0:2, 0:1])
                # ---- early collective: sums + global -min/max ----
                gather(cstatS, cgathS, SUMS_N, 0)
                nc.scalar.dma_start(
                    out=gthS[:], in_=cgathS[:, 0:SUMS_N].transpose([1, 0]))
                nc.scalar.activation(scrA[:], gthS[0:2 * T, :], AF.Copy,
                                     accum_out=sv2[:])
                nc.scalar.dma_start(
                    out=ggrow[0:1, :],
                    in_=cgathS[:, 2 * T:SUMS_N].transpose([1, 0]))

                # ---- min-side lookups -> wave-1 collective (hidden under
                # the max-side lookups) ----
                for t in range(T):
                    box_red(hn32, int(x1s[t]), int(x2s[t]), rmm[:, t:t + 1])
                nc.vector.tensor_tensor(stk[:, 0:T], rmm[:, 0:T],
                                        rinfx2C[:, 0:T], alu.add)
                nc.tensor.transpose(stkTn[:], stk[:, 0:T], identC)
                nc.vector.tensor_reduce(bmv[0:T, 0:1], stkTn[:], X, alu.max)
                nc.sync.dma_start(out=cstatN[0:1, :], in_=bmv[0:T, 0:1])
                gather(cstatN, cgathN, T, 0)
                nc.scalar.dma_start(out=gtm2[:, 0:NCORES],
                                    in_=cgathN[:, :].transpose([1, 0]))

                # ---- max-side lookups -> wave-2 collective ----
                for t in range(T):
                    box_red(g32, int(x1s[t]), int(x2s[t]),
                            rmm[:, T + t:T + t + 1])
                nc.vector.tensor_tensor(stk[:, T:2 * T], rmm[:, T:2 * T],
                                        rinfx2C[:, T:2 * T], alu.add)
                nc.tensor.transpose(stkTx[:], stk[:, T:2 * T], identC)
                nc.vector.tensor_reduce(bmv[T:2 * T, 0:1], stkTx[:], X,
                                        alu.max)
                nc.sync.dma_start(out=cstatX[0:1, :], in_=bmv[T:2 * T, 0:1])
                gather(cstatX, cgathX, T, 0)

                # ---- sums finish + loss_acc pipeline (hidden under the
                # wave-2 collective hops) ----
                nc.vector.tensor_scalar_mul(meanv[:], sv2[0:T, 0:1], cntinvC)
                nc.vector.tensor_scalar_mul(m2sv[:], sv2[0:T, 0:1], meanv[:])
                nc.vector.tensor_scalar(varv[:], sv2[T:2 * T, 0:1], m2sv[:],
                                        cm1invC, alu.subtract, alu.mult)
                nc.scalar.sqrt(stdv[:], varv[:])
                nc.tensor.transpose(meanT_p[:], meanv[:], ident32C)
                nc.scalar.copy(meanTS[:], meanT_p[:])
                nc.tensor.matmul(mr_p[:], onesrowC, meanTS[:],
                                 start=True, stop=True)
                nc.vector.tensor_reduce(ggred[0:1, 0:1],
                                        ggrow[0:1, 0:2 * NCORES:2], X,
                                        alu.max)
                nc.vector.tensor_reduce(ggred[0:1, 1:2],
                                        ggrow[0:1, 1:2 * NCORES:2], X,
                                        alu.max)
                nc.vector.tensor_tensor(aden[:], ggred[0:1, 1:2],
                                        ggred[0:1, 0:1], alu.add)
                nc.vector.reciprocal(arecip[:], aden[:])
                nc.gpsimd.partition_broadcast(acolS[:], arecip[:])
                nc.vector.tensor_scalar(qm[:], mr_p[:], meanv[:], acolS[:],
                                        alu.subtract, alu.mult)
                nc.vector.tensor_tensor(t2m[:], gmatC, qm[:], alu.subtract)
                nc.scalar.activation(t3m[:], t2m[:], AF.Relu,
                                     accum_out=raccv[:])
                nc.tensor.matmul(pl2[:, 0:1], raccv[:], ones32C,
                                 start=True, stop=True)

                # ---- wave-2 readback + final ----
                nc.scalar.dma_start(out=gtm2[:, NCORES:2 * NCORES],
                                    in_=cgathX[:, :].transpose([1, 0]))
                nc.vector.tensor_reduce(
                    redM[:, 0:1], gtm2[:, 0:NCORES], X, alu.max)
                nc.vector.tensor_reduce(
                    redM[:, 1:2], gtm2[:, NCORES:2 * NCORES], X, alu.max)
                nc.vector.tensor_tensor(rngv[:], redM[:, 1:2], redM[:, 0:1],
                                        alu.add)
                nc.vector.reciprocal(rinv[:], rngv[:])
                nc.vector.tensor_tensor(srv[:], stdv[:], rinv[:], alu.mult)
                nc.tensor.matmul(pl2[:, 1:2], srv[:], ones32C,
                                 start=True, stop=True)
                nc.scalar.activation(out3[:, 0:2], pl2[:], AF.Copy,
                                     accum_out=out3[:, 2:3])
                nc.sync.dma_start(out=out[:], in_=out3[0:1, 0:3])

    nc.compile()
    return nc


def kernel(d_pred, bboxes, _trace=False):
    from concourse.bass_utils import run_bass_kernel_spmd

    d_pred = np.asarray(d_pred, dtype=np.float32)
    bboxes = np.asarray(bboxes, dtype=np.int32)
    depth = d_pred[0, 0]
    x1, y1, x2, y2 = (bboxes[:, i].astype(np.int64) for i in range(4))

    cnt = ((x2 - x1) * (y2 - y1)).astype(np.float64)
    cntinv = (1.0 / cnt).astype(np.float32)
    cm1inv = (1.0 / (cnt - 1.0)).astype(np.float32)

    ii = np.arange(T)[:, None]
    jj = np.arange(T)[None, :]
    gmat = np.where(jj > ii, (jj - ii) / float(T), -BIG).astype(np.float32)

    rows = np.arange(H)
    rind_full = ((rows[:, None] >= y1[None, :])
                 & (rows[:, None] < y2[None, :])).astype(np.float32)

    cols = np.arange(W)
    colind_full = ((cols[:, None] >= x1[None, :])
                   & (cols[:, None] < x2[None, :])).astype(np.float16)

    in_maps = []
    for c in range(NCORES):
        dloc = depth[c * R:(c + 1) * R]                       # [128, 2048]
        ri = rind_full[c * R:(c + 1) * R]                     # [128, 32]
        rinfx = np.where(ri > 0, 0.0, -BIG).astype(np.float32)

        cstc = np.zeros((128, CST_W), np.float32)
        cstc[:, 0:128] = np.eye(128, dtype=np.float32)
        cstc[0:T, 128:160] = gmat
        cstc[0:T, 160] = cntinv
        cstc[0:T, 161] = cm1inv
        cstc[:, 162] = 1.0
        cstc[0, 163:163 + T] = 1.0
        cstc[:, 196:228] = rinfx
        cstc[:, 228:260] = rinfx
        cstc[:, 260:292] = ri

        dsT_h = dloc.T.reshape(NCHUNK, 128, R).transpose(1, 0, 2) \
            .reshape(128, W).astype(np.float16)
        cold_h = colind_full.reshape(NCHUNK, 128, T).transpose(1, 0, 2) \
            .reshape(128, NCHUNK * T)

        dinH = np.empty((R, DINH_W), np.float16)
        dinH[:, 0:W] = (-dloc).astype(np.float16)
        dinH[:, W:2 * W] = dloc.astype(np.float16)
        dinH[:, 2 * W:3 * W] = dsT_h
        dinH[:, 3 * W:DINH_W] = cold_h
        in_maps.append({"dinH": dinH, "cst": cstc})

    nc = _build_program(bboxes)
    res = run_bass_kernel_spmd(nc, in_maps, list(range(NCORES)),
                               trace=_trace)
    o = res.results[0]["out"].astype(np.float32)
    outs = (o[0:1].copy(), o[1:2].copy(), o[2:3].copy())
    if _trace:
        return outs, res
    return outs
